# revision 38
# baseline (speedup 1.0000x reference)
"""Bidirectional-ALiBi bias kernel for Trainium2 (Bass/Tile), 8-core SPMD.

Computes out[h, i, j] = |j - i| * m where m = alpha[h] on the first
row/column, gamma[h] above the diagonal, beta[h] below it, and 0 on the
(non-edge) diagonal.  Output [16, 2048, 2048] f32, sharded 2 heads/core.

Strategy: every interior row i is a shifted window of the per-head
profile V(k) = gamma*max(k,0) + beta*max(-k,0), k = j - i.  Each core
computes, per head, overlapping 2047-col diagonalized tiles
W[p, c-lo] = V(c - p - 2047) with lo in {0, 1024, 2048}, chosen so that
for every 128-row block t:
  right half (cols 1024..2047) = ONE contiguous slice of W_B (t>=8)
      or W_C (t<8)  -> DMA'd directly, zero copies;
  left half (cols 0..1023) = one slice of W_A/W_B/W_C plus col 0
      (alpha*i) -> assembled into a QL tile, then DMA'd.
Every DMA descriptor is >= 1 KB and all but two are 3-8 KB: HBM write
efficiency at 8 KB row stride is set by descriptor size (4-8 KB
descriptors sustain ~420 GB/s; sub-1 KB pay a DRAM row-activation per
row, <512 B an SDMA read-modify-write, collapsing throughput).

Sign-region specialization (cuts the critical path and engine load):
k's sign is uniform over most regions, so
  k>0 everywhere  -> V = relu(gamma*k): ACT writes the W tile slice
      directly; no beta, no DVE op needed (chunk B cols [1279,2047),
      chunk C cols [1023,2047) - which is ALL of rights t=0..7);
  k<0 everywhere  -> V = (-beta)*k: one DVE multiply from K (chunk B
      cols [0,1023), all of chunk A - so lefts t=8..15 skip T2/W);
  mixed           -> T2 = relu(gamma*k) on ACT, W = max(-beta*k, T2).
The first DMA is therefore gated only by the gamma broadcast-DMA
semaphore (~10.3us), a 768-col iota slice, and one ACT relu: first
bytes at ~12.6us, stream dead-flat at 412-420 GB/s to ~95us, ~97.9us
total (from a 125.5us starting point).

Engine placement (respects the DVE/GpSimd shared-SBUF-port lock, and
keeps both HWDGE rings' trigger queues self-paced):
  gpsimd: gamma SWDGE load, master K iota (3 slices), IB iota, memsets
  ACT (nc.scalar): K chunk derives (K +- 1024), all relu work,
      right-half DMA triggers
  DVE: mixed-region max ops, k<0 multiplies, left-half assembly
  SP ring (nc.sync): beta/alpha loads + left-half DMA triggers
Left and right streams are 16.8 MB each - the rings stay balanced.

Hardware notes (from NTFF profiling): the 16 SDMA engines sustain
419.5 GB/s aggregate (96% of the 435 GB/s SBUF-AXI fabric; the rest is
per-descriptor metadata + per-packet overhead) and run 100% busy for
the whole stream, so 33.6 MB/core takes 80.0 us flat.  Fixed costs:
~6.5 us NEFF start barrier, ~2.9 us end barrier.  Coefficient
partition-broadcast DMAs (128 descriptors, all 16 engines) complete
~3.8 us after the barrier; a minimal 2-descriptor DMA's semaphore
was measured to arrive ~9 us late - never use tiny coefficient DMAs.
GpSimd and DVE 2-port perf-mode ops fully block each other (shared
SBUF port pair) - putting elementwise work on gpsimd 2x'd the kernel.
"""

import numpy as np

H = 16
S = 2048
P = 128
N_CORES = 8
H_LOC = H // N_CORES  # 2 heads per core
NT = S // P  # 16 row blocks per head
HW = 1024  # half-row width
CW = 2047  # chunk width

# chunk lo offsets: A=[0,2047), B=[1024,3071), C=[2048,4095)
LO_A, LO_B, LO_C = 0, 1024, 2048
# W_B k-sign region boundaries (tile-local cols): [0,S3)=k<0 (mul),
# [S3,S2)=mixed (max), [S2,CW)=k>0 (relu)
S3, S2 = 1023, 1279
# W_C regions: [0,CM)=mixed, [CM,CW)=k>0 (c >= 2175 > p + 2047 always)
CM = 127

_NC = None


def _build():
    import concourse.bacc as bacc
    import concourse.mybir as mybir
    from concourse.tile import TileContext

    f32 = mybir.dt.float32
    Copy = mybir.ActivationFunctionType.Copy
    Ident = mybir.ActivationFunctionType.Identity
    Relu = mybir.ActivationFunctionType.Relu
    mult, amax = mybir.AluOpType.mult, mybir.AluOpType.max
    nc = bacc.Bacc("TRN2", target_bir_lowering=False, debug=False)

    alpha_d = nc.dram_tensor("alpha", [H_LOC], f32, kind="ExternalInput").ap()
    beta_d = nc.dram_tensor("beta", [H_LOC], f32, kind="ExternalInput").ap()
    gamma_d = nc.dram_tensor("gamma", [H_LOC], f32, kind="ExternalInput").ap()
    out_d = nc.dram_tensor("out", [H_LOC, S, S], f32, kind="ExternalOutput").ap()

    # left half of block t reads c in [2048-128t, 3071-128t); right half
    # c in [3071-128t, 4095-128t).
    left_lo = lambda t: LO_C if t == 0 else (LO_B if t < 8 else LO_A)
    right_lo = lambda t: LO_C if t < 8 else LO_B

    with TileContext(nc) as tc:
        with (
            tc.tile_pool(name="coef", bufs=1) as cpool,
            tc.tile_pool(name="kpool", bufs=3) as kpool,
            tc.tile_pool(name="tpool", bufs=1) as tpool,
            tc.tile_pool(name="wpool", bufs=1) as wpool,
            tc.tile_pool(name="qlpool", bufs=8) as qlpool,
            tc.tile_pool(name="qrpool", bufs=1) as qrpool,
        ):
            # --- coefficient loads: partition-broadcast HWDGE DMAs (128
            # descriptors on all 16 engines -> semaphore ~10.3us; SWDGE
            # and minimal-descriptor variants were both measured slower).
            G2 = cpool.tile([P, H_LOC], f32, tag="G2")
            nc.sync.dma_start(out=G2[:], in_=gamma_d.partition_broadcast(P))
            B2 = cpool.tile([P, H_LOC], f32, tag="B2")
            nc.scalar.dma_start(out=B2[:], in_=beta_d.partition_broadcast(P))
            A2 = cpool.tile([P, H_LOC], f32, tag="A2")
            nc.sync.dma_start(out=A2[:], in_=alpha_d.partition_broadcast(P))

            # --- master K iota in three slices: k>0 slice first (feeds the
            # first relu->DMA chain), then the k<0 slice (feeds the mul ops
            # and left t=8, seeding the SP ring), mixed slice last.
            # K_B[p, x] = (LO_B + x) - p - 2047
            KB = kpool.tile([P, CW], f32, tag="K")
            for x0, x1 in ((S2, CW), (0, S3), (S3, S2)):
                nc.gpsimd.iota(
                    KB[:, x0:x1],
                    pattern=[[1, x1 - x0]],
                    base=LO_B + x0 - (S - 1),
                    channel_multiplier=-1,
                    allow_small_or_imprecise_dtypes=True,
                )
            IB = cpool.tile([P, NT], f32, tag="IB")
            nc.gpsimd.iota(
                IB[:],
                pattern=[[P, NT]],
                base=0,
                channel_multiplier=1,
                allow_small_or_imprecise_dtypes=True,
            )
            bias_p = cpool.tile([P, 1], f32, tag="bias_p")
            nc.gpsimd.memset(bias_p[:], float(HW))
            bias_n = cpool.tile([P, 1], f32, tag="bias_n")
            nc.gpsimd.memset(bias_n[:], float(-HW))

            NB2 = cpool.tile([P, H_LOC], f32, tag="NB2")
            nc.vector.tensor_scalar_mul(NB2[:], B2[:], -1.0)
            Rs = {}
            for h in range(H_LOC):
                Rh = cpool.tile([P, NT], f32, tag=f"Rs{h}")
                nc.vector.tensor_scalar_mul(Rh[:], IB[:], A2[:, h : h + 1])
                Rs[h] = Rh

            Ks = {LO_B: KB}
            Ws = {}  # (h, lo) -> W tile (chunk A has none: pure k<0)

            def derive_k(lo, bias):  # ACT: K_lo = K_B + (lo - LO_B)
                Kg = kpool.tile([P, CW], f32, tag="K")
                nc.scalar.activation(out=Kg[:], in_=KB[:], func=Ident, bias=bias[:])
                Ks[lo] = Kg

            def w_tile(h, lo):
                Wt = Ws.get((h, lo))
                if Wt is None:
                    Wt = wpool.tile([P, CW], f32, tag=f"W{h}{lo}")
                    Ws[(h, lo)] = Wt
                return Wt

            def relu_into_w(h, lo, x0, x1):  # ACT: W[x0:x1] = relu(gamma*K)
                nc.scalar.activation(
                    out=w_tile(h, lo)[:, x0:x1],
                    in_=Ks[lo][:, x0:x1],
                    func=Relu,
                    scale=G2[:, h : h + 1],
                )

            def t2_slice(h, lo, x0, x1):  # ACT: T2 = relu(gamma*K), scratch
                T2t = tpool.tile([P, S3], f32, tag=f"T2{h}")
                nc.scalar.activation(
                    out=T2t[:, : x1 - x0],
                    in_=Ks[lo][:, x0:x1],
                    func=Relu,
                    scale=G2[:, h : h + 1],
                )
                return T2t

            def max_into_w(h, lo, x0, x1, T2t):  # DVE: W = max(-beta*K, T2)
                nc.vector.scalar_tensor_tensor(
                    out=w_tile(h, lo)[:, x0:x1],
                    in0=Ks[lo][:, x0:x1],
                    scalar=NB2[:, h : h + 1],
                    in1=T2t[:, : x1 - x0],
                    op0=mult,
                    op1=amax,
                )

            def mul_into_w(h, lo, x0, x1):  # DVE: W = (-beta)*K  (k<0 region)
                nc.vector.tensor_scalar_mul(
                    w_tile(h, lo)[:, x0:x1],
                    Ks[lo][:, x0:x1],
                    NB2[:, h : h + 1],
                )

            def emit_right(h, t, j0=HW, j1=S, ring=None):
                lo = right_lo(t)
                x = (3071 - 128 * t - lo) + (j0 - HW)
                (ring or nc.scalar).dma_start(
                    out=out_d[h, P * t : P * (t + 1), j0:j1],
                    in_=Ws[(h, lo)][:, x : x + (j1 - j0)],
                )

            QR0s = {}

            def build_right0(h):
                # block 0 right half: row 0 must read alpha*j -> assemble.
                # All ops on ACT so the row-0 overwrite is queue-ordered.
                QR = qrpool.tile([P, HW], f32, tag=f"QR{h}")
                nc.scalar.activation(
                    out=QR[:], in_=Ws[(h, LO_C)][:, S3 : S3 + HW], func=Copy
                )
                nc.scalar.activation(
                    out=QR[0:1, :],
                    in_=Ks[LO_C][0:1, S3 : S3 + HW],
                    func=Copy,
                    scale=A2[0:1, h : h + 1],
                )
                QR0s[h] = QR

            def trig_right0(h):
                nc.scalar.dma_start(out=out_d[h, 0:P, HW:S], in_=QR0s[h][:])

            def emit_left(h, t):
                # col 0 = alpha*i; cols 1..1023 from the serving chunk.
                # t=0: row 0 = alpha*j.  t>=8: pure k<0 -> multiply from K_A
                # directly (chunk A has no W tile at all).
                # t=8 reads c in [1024, 2047) == chunk B's k<0 region, so it
                # can be built from K_B the moment the iota lands (~12.7us),
                # seeding the SP ring before anything else is ready.
                lo = LO_B if t == 8 else left_lo(t)
                a = 2048 - 128 * t - lo  # tile-local col of j=1
                QL = qlpool.tile([P, HW], f32, tag=f"QL{h}")
                if t >= 8:
                    nc.vector.tensor_scalar_mul(
                        QL[:, 1:HW], Ks[lo][:, a : a + HW - 1], NB2[:, h : h + 1]
                    )
                else:
                    nc.vector.tensor_copy(
                        out=QL[:, 1:HW], in_=Ws[(h, lo)][:, a : a + HW - 1]
                    )
                if t == 0:
                    nc.vector.tensor_scalar_mul(
                        QL[0:1, 1:HW],
                        Ks[lo][0:1, a : a + HW - 1],
                        A2[0:1, h : h + 1],
                    )
                nc.vector.tensor_copy(out=QL[:, 0:1], in_=Rs[h][:, t : t + 1])
                nc.sync.dma_start(out=out_d[h, P * t : P * (t + 1), 0:HW], in_=QL[:])

            # --- schedule (code order == per-engine queue order) ---
            # B chunk, h0: k>0 slice via ACT relu straight into W_B, DMA the
            # matching piece of right t=8 immediately; mixed + k<0 slices
            # follow on DVE.
            relu_into_w(0, LO_B, S2, CW)
            emit_right(0, 8, HW + S2 - S3, S)  # j in [1280, 2048), 3 KB rows
            relu_into_w(1, LO_B, S2, CW)
            emit_right(1, 8, HW + S2 - S3, S)
            # k<0 multiplies + left t=8 first on DVE: they only need the K
            # iota, while the max ops wait on ACT's T2 -- emitting the max
            # first was measured to stall DVE (in-order queue) ~3us.
            mul_into_w(0, LO_B, 0, S3)
            emit_left(0, 8)  # from K_B directly -- earliest possible left
            mul_into_w(1, LO_B, 0, S3)
            emit_left(1, 8)
            T2 = t2_slice(0, LO_B, S3, S2)
            max_into_w(0, LO_B, S3, S2, T2)
            emit_right(0, 8, HW, HW + S2 - S3)  # j in [1024, 1280), 1 KB rows
            T2 = t2_slice(1, LO_B, S3, S2)
            max_into_w(1, LO_B, S3, S2, T2)
            emit_right(1, 8, HW, HW + S2 - S3)
            for t in range(9, NT):  # rights t=9..15 h0 (ACT ring, seeds it)
                emit_right(0, t)
            for t in range(1, 8):  # lefts t=1..7 h0 (SP ring) -- pumps the
                emit_left(0, t)  # second ring during the ramp
            # C chunk: the k>0 relu region [CM, CW) serves ALL of rights
            # t=0..7 (they read tile cols >= 1023-128*7 = 127 = CM); the
            # tiny mixed region [0, CM) only serves left t=0.
            # tile_wait_until stops the scheduler from hoisting the 2us
            # derive ahead of the first relu->DMA chain on the ACT queue
            # (it models the gamma DMA as slower than the K iota and
            # otherwise reorders, delaying first bytes by ~3us).
            with tc.tile_wait_until(0.005):
                derive_k(LO_C, bias_p)
            # ALL remaining ACT compute goes here, BEFORE the bulk of the
            # right-half triggers: ring-FIFO depth drain-paces every ACT
            # instruction sitting behind queued triggers, and compute
            # stuck there was measured to starve the left stream at ~70us.
            relu_into_w(0, LO_C, CM, CW)
            relu_into_w(1, LO_C, CM, CW)
            # lefts t=1..7 h1, then chunk A (pure k<0): K_A derived on DVE
            # (immediate scalar add, no ACT queue involvement) feeds lefts
            # t=8..15 so the SP ring never runs dry mid-stream.  The C
            # mixed-region ops and left t=0 come last on DVE: they wait on
            # ACT's T2c and must not block the ready left-half assembly.
            for t in range(1, 8):
                emit_left(1, t)
            KA = kpool.tile([P, CW], f32, tag="K")
            nc.vector.tensor_scalar_add(KA[:], KB[:], float(LO_A - LO_B))
            Ks[LO_A] = KA
            for t in range(9, NT):  # t=8 was emitted early from K_B
                emit_left(0, t)
            for t in range(9, NT):
                emit_left(1, t)
            T2 = t2_slice(0, LO_C, 0, CM)
            max_into_w(0, LO_C, 0, CM, T2)
            T2 = t2_slice(1, LO_C, 0, CM)
            max_into_w(1, LO_C, 0, CM, T2)
            build_right0(0)
            build_right0(1)
            emit_left(0, 0)
            emit_left(1, 0)
            # remaining right-half triggers (drain-paced is fine here).  The
            # last two ride the SP ring: the ACT ring was measured to finish
            # ~5us after the SP ring, leaving the final stretch single-queue.
            for t in range(9, NT):  # rights t=9..15 h1
                emit_right(1, t)
            for t in range(1, 8):
                emit_right(0, t)
            trig_right0(0)
            for t in range(3, 8):
                emit_right(1, t)
            trig_right0(1)
            emit_right(1, 1, ring=nc.sync)
            emit_right(1, 2, ring=nc.sync)

    nc.compile()
    return nc


def _run(alpha, beta, gamma, **spmd_kwargs):
    """Compile (cached) and run on the 8 NeuronCores; returns BassKernelResults."""
    global _NC
    if _NC is None:
        _NC = _build()
    from concourse import bass_utils

    alpha = np.ascontiguousarray(alpha, dtype=np.float32)
    beta = np.ascontiguousarray(beta, dtype=np.float32)
    gamma = np.ascontiguousarray(gamma, dtype=np.float32)
    in_maps = [
        {
            "alpha": alpha[c * H_LOC : (c + 1) * H_LOC],
            "beta": beta[c * H_LOC : (c + 1) * H_LOC],
            "gamma": gamma[c * H_LOC : (c + 1) * H_LOC],
        }
        for c in range(N_CORES)
    ]
    return bass_utils.run_bass_kernel_spmd(
        _NC, in_maps, core_ids=list(range(N_CORES)), **spmd_kwargs
    )


def kernel(alpha, beta, gamma, seq_len):
    assert int(seq_len) == S, f"kernel hardcodes seq_len={S}, got {seq_len}"
    res = _run(alpha, beta, gamma)
    return np.concatenate([r["out"] for r in res.results], axis=0)
